# revision 53
# baseline (speedup 1.0000x reference)
"""Trainium2 Bass kernel for nn_BertAttention_90297392431744.

Sharding: 8 cores = (4 batches) x (2 head-groups of 6 heads).
Each core computes, for its batch b and heads hg*6..hg*6+5:
  - positioned hs/ctx (vis_position added to first feat_len rows)
  - Q/K/V projections (fp32r matmuls)
  - masked, gated scores  -> DRAM (full [6, 1024, 1024] slice)
  - softmax (no max-subtraction; exact handling of fully-masked rows) + PV
Structured-mask sparsity: rows q < 511 keep only cols {q-1, q}; row 511 keeps
all; rows q >= 512 keep only [feat_len-1, 512). Uncomputed regions of the
scores output are written from a constant NEG tile with replicated-source
DMAs.
"""

import sys

for _p in ("/opt/trn_rl_repo",):
    if _p not in sys.path:
        sys.path.insert(0, _p)

import numpy as np

import concourse.bass as bass
import concourse.bacc as bacc
import concourse.mybir as mybir
import concourse.tile as tile
from concourse.masks import make_identity

dt = mybir.dt
Alu = mybir.AluOpType
Act = mybir.ActivationFunctionType

B, S, H = 4, 1024, 768
NH, DH = 12, 64
NCORES = 8
HPC = 6            # heads per core
FPC = HPC * DH     # 384 features per core
NEG = -100000.0
PROX = 512

# per-qc computed-window geometry: (kstart, w)
QC_WIN = {0: (0, 128), 1: (0, 256), 2: (128, 256), 3: (0, 1024)}


def _reorder(ap, order):
    return bass.AP(ap.tensor, ap.offset, [ap.ap[i] for i in order])


def _build(use_amask: bool):
    nc = bacc.Bacc("TRN2", target_bir_lowering=False, debug=True)
    f32, f32r, i32, u8 = dt.float32, dt.float32r, dt.int32, dt.uint8

    hs_d = nc.dram_tensor("hs", [S, H], f32, kind="ExternalInput")
    ctx_d = nc.dram_tensor("ctx", [S, H], f32, kind="ExternalInput")
    vis_d = nc.dram_tensor("vis", [S, H], f32, kind="ExternalInput")
    gate_d = nc.dram_tensor("gate", [S, S], f32, kind="ExternalInput")
    am_d = nc.dram_tensor("amask", [1, S], f32, kind="ExternalInput")
    wq_d = nc.dram_tensor("Wq", [FPC, H], f32, kind="ExternalInput")
    wk_d = nc.dram_tensor("Wk", [FPC, H], f32, kind="ExternalInput")
    wv_d = nc.dram_tensor("Wv", [FPC, H], f32, kind="ExternalInput")
    bq_d = nc.dram_tensor("bq", [1, FPC], f32, kind="ExternalInput")
    bk_d = nc.dram_tensor("bk", [1, FPC], f32, kind="ExternalInput")
    bv_d = nc.dram_tensor("bv", [1, FPC], f32, kind="ExternalInput")
    fl_d = nc.dram_tensor("fl", [1, 1], i32, kind="ExternalInput")

    sc_d = nc.dram_tensor("scores", [HPC, S, S], f32, kind="ExternalOutput")
    out_d = nc.dram_tensor("out", [S, FPC], f32, kind="ExternalOutput")

    import contextlib

    with tile.TileContext(nc) as tc, contextlib.ExitStack() as stack:
        cpool = stack.enter_context(tc.tile_pool(name="const", bufs=1))
        qkv = stack.enter_context(tc.tile_pool(name="qkv", bufs=1))

        # ---------------- constants ----------------
        ident = cpool.tile([128, 128], f32)
        make_identity(nc, ident[:])
        ident_r = cpool.tile([128, 128], f32r)
        nc.scalar.copy(ident_r[:], ident[:])
        ones_row = cpool.tile([1, 128], f32)
        nc.vector.memset(ones_row[:], 1.0)
        neg = cpool.tile([128, 1024], f32)
        nc.gpsimd.memset(neg[:], NEG)
        negreg = nc.gpsimd.to_reg(NEG)

        fl_i = cpool.tile([1, 1], i32)
        nc.gpsimd.dma_start(fl_i[:], fl_d[:])
        fl_f = cpool.tile([1, 1], f32)
        nc.vector.tensor_copy(fl_f[:], fl_i[:])

        flb = cpool.tile([128, 1], f32)
        flm1b = cpool.tile([128, 1], f32)
        rowmask = cpool.tile([128, 8], f32)
        fpred = cpool.tile([128, 512], u8)
        fpred2 = cpool.tile([128, 1024], u8)

        gpool = stack.enter_context(tc.tile_pool(name="gatep", bufs=1))
        prep1 = contextlib.ExitStack()
        tp = prep1.enter_context(tc.tile_pool(name="tpose", bufs=1))
        wtvp = prep1.enter_context(tc.tile_pool(name="wtv", bufs=1))
        prep2 = contextlib.ExitStack()
        wtp = prep2.enter_context(tc.tile_pool(name="wtqk", bufs=1))

        # ============ phase A: loads, masks, weight transposes, input transposes
        with tc.tile_pool(name="ppsA", bufs=4, space="PSUM") as pps, \
             tc.tile_pool(name="wraw", bufs=3) as wraw, \
             tc.tile_pool(name="raw", bufs=4) as rawp:

            # fl broadcast [128,1] via PE ones-matmul
            ps_fl = pps.tile([128, 512], f32, tag="pp")
            nc.tensor.matmul(ps_fl[:, 0:1], ones_row[:], fl_f[:], start=True, stop=True)
            nc.scalar.copy(flb[:], ps_fl[:, 0:1])
            nc.vector.tensor_scalar(flm1b[:], flb[:], -1.0, None, op0=Alu.add)

            # row mask [128, 8]: (p + 128*j) < feat_len
            iota_p = rawp.tile([128, 8], i32, tag="iotap")
            nc.gpsimd.iota(iota_p[:], pattern=[[128, 8]], base=0, channel_multiplier=1)
            iota_pf = rawp.tile([128, 8], f32, tag="iotapf")
            nc.vector.tensor_copy(iota_pf[:], iota_p[:])
            nc.vector.tensor_scalar(rowmask[:], iota_pf[:], flb[:], None, op0=Alu.is_lt)

            # feat predicate [128, 512] uint8: k < feat_len - 1
            iota_k = rawp.tile([128, 512], i32, tag="iotak")
            nc.gpsimd.iota(iota_k[:], pattern=[[1, 512]], base=0, channel_multiplier=0)
            iota_kf = rawp.tile([128, 512], f32, tag="iotakf")
            nc.vector.tensor_copy(iota_kf[:], iota_k[:])
            nc.vector.tensor_scalar(fpred[:], iota_kf[:], flm1b[:], None, op0=Alu.is_lt)
            s1 = fpred[:]
            srep = bass.AP(s1.tensor, s1.offset, [s1.ap[0], [0, 2], [1, 512]])
            drep = fpred2[:]
            drep = bass.AP(drep.tensor, drep.offset, [drep.ap[0], [512, 2], [1, 512]])
            nc.gpsimd.dma_start(drep, srep)

            if use_amask:
                am_sb = cpool.tile([1, 1024], f32)
                nc.sync.dma_start(am_sb[:], am_d[:])
                am_b = cpool.tile([128, 1024], f32)
                for n in range(2):
                    ps_am = pps.tile([128, 512], f32, tag="pp")
                    nc.tensor.matmul(ps_am[:], ones_row[:], am_sb[0:1, 512 * n:512 * n + 512],
                                     start=True, stop=True)
                    nc.scalar.copy(am_b[:, 512 * n:512 * n + 512], ps_am[:])

            # weights: load + transpose
            wts = {}
            for wname, w_d in (("q", wq_d), ("k", wk_d), ("v", wv_d)):
                wr = []
                for m in range(3):
                    t = wraw.tile([128, H], f32, tag="wraw", name=f"w{wname}{m}")
                    nc.sync.dma_start(t[:], w_d[128 * m:128 * m + 128, :])
                    wr.append(t)
                wt_i = []
                for i in range(6):
                    ps = pps.tile([128, 512], f32, tag="pp")
                    for m in range(3):
                        nc.tensor.transpose(ps[:, 128 * m:128 * m + 128],
                                            wr[m][:, 128 * i:128 * i + 128], ident[:])
                    t = (wtvp if wname == "v" else wtp).tile([128, FPC], f32r,
                                                             name=f"wt{wname}{i}")
                    nc.scalar.copy(t[:], ps[:, 0:FPC])
                    wt_i.append(t)
                wts[wname] = wt_i

            # bias columns [128,1] x3 via SBUF->SBUF dma reshape (gpsimd queue)
            bcols = {}
            for bname, b_d in (("q", bq_d), ("k", bk_d), ("v", bv_d)):
                b_sb = cpool.tile([1, FPC], f32, name=f"b{bname}sb")
                nc.gpsimd.dma_start(b_sb[:], b_d[:])
                cols = []
                for m in range(3):
                    c = cpool.tile([128, 1], f32, name=f"b{bname}c{m}")
                    nc.gpsimd.dma_start(c[:, 0:1], b_sb[0:1, 128 * m:128 * m + 128])
                    cols.append(c)
                bcols[bname] = (b_sb, cols)
            for m in range(3):  # bq -> bq/8
                c = bcols["q"][1][m]
                nc.vector.tensor_scalar(c[:], c[:], 0.125, None, op0=Alu.mult)

            # hs/ctx/vis: load (trio-interleaved), positioned add, transpose
            hsT = [tp.tile([128, 1024], f32r, name=f"hsT{i}") for i in range(6)]
            ctxT = [tp.tile([128, 1024], f32r, name=f"ctxT{i}") for i in range(6)]
            for v in range(2):
                hts, cts = [], []
                for u in range(4):
                    j = 4 * v + u
                    tv = rawp.tile([128, H], f32, tag="visraw", name=f"visr{j}")
                    nc.sync.dma_start(tv[:], vis_d[128 * j:128 * j + 128, :])
                    th = rawp.tile([128, H], f32, tag="hsraw", name=f"hsr{j}")
                    nc.sync.dma_start(th[:], hs_d[128 * j:128 * j + 128, :])
                    tcx = rawp.tile([128, H], f32, tag="ctxraw", name=f"ctxr{j}")
                    nc.sync.dma_start(tcx[:], ctx_d[128 * j:128 * j + 128, :])
                    nc.vector.scalar_tensor_tensor(th[:], in0=tv[:], scalar=rowmask[:, j:j + 1],
                                                   in1=th[:], op0=Alu.mult, op1=Alu.add)
                    nc.vector.scalar_tensor_tensor(tcx[:], in0=tv[:], scalar=rowmask[:, j:j + 1],
                                                   in1=tcx[:], op0=Alu.mult, op1=Alu.add)
                    hts.append(th)
                    cts.append(tcx)
                for i in range(6):
                    ps = pps.tile([128, 512], f32, tag="pp")
                    for u in range(4):
                        nc.tensor.transpose(ps[:, 128 * u:128 * u + 128],
                                            hts[u][:, 128 * i:128 * i + 128], ident[:])
                    nc.scalar.copy(hsT[i][:, 512 * v:512 * v + 512], ps[:])
                    ps2 = pps.tile([128, 512], f32, tag="pp")
                    for u in range(4):
                        nc.tensor.transpose(ps2[:, 128 * u:128 * u + 128],
                                            cts[u][:, 128 * i:128 * i + 128], ident[:])
                    nc.scalar.copy(ctxT[i][:, 512 * v:512 * v + 512], ps2[:])

        # ============ gate loads (after input streams in SP FIFO)
        gates = []
        for qc in range(4):
            ks, w = QC_WIN[qc]
            g = gpool.tile([128, w], f32, name=f"gate{qc}")
            nc.sync.dma_start(g[:], gate_d[128 * qc:128 * qc + 128, ks:ks + w])
            gates.append(g)
        gate_blk = gpool.tile([128, 2048], f32)
        d = gate_d[512:1024, 0:512]
        srcap = bass.AP(d.tensor, d.offset, [[1024, 128], [128 * 1024, 4], [1, 512]])
        dstap = gate_blk[:, :]
        dstap = bass.AP(dstap.tensor, dstap.offset, [dstap.ap[0], [512, 4], [1, 512]])
        nc.sync.dma_start(dstap, srcap)

        # ============ phase C1: QT/KT projections (m-interleaved)
        QT = [qkv.tile([128, 1024], f32r, name=f"QT{m}") for m in range(3)]
        KT = [qkv.tile([128, 1024], f32r, name=f"KT{m}") for m in range(3)]
        V = [qkv.tile([128, FPC], f32, name=f"V{j}") for j in range(8)]
        proj_anchor = []
        with tc.tile_pool(name="ppsC", bufs=2, space="PSUM") as pps:
            for m in [0]:
                for n in range(2):
                    psq = pps.tile([128, 512], f32, tag="pp")
                    for i in range(6):
                        nc.tensor.matmul(psq[:], wts["q"][i][:, 128 * m:128 * m + 128],
                                         hsT[i][:, 512 * n:512 * n + 512],
                                         start=(i == 0), stop=(i == 5))
                    qa = nc.scalar.activation(QT[m][:, 512 * n:512 * n + 512], psq[:],
                                              Act.Identity, bias=bcols["q"][1][m][:], scale=0.125)
                    proj_anchor.append(qa)
                    psk = pps.tile([128, 512], f32, tag="pp")
                    for i in range(6):
                        nc.tensor.matmul(psk[:], wts["k"][i][:, 128 * m:128 * m + 128],
                                         ctxT[i][:, 512 * n:512 * n + 512],
                                         start=(i == 0), stop=(i == 5))
                    ka = nc.scalar.activation(KT[m][:, 512 * n:512 * n + 512], psk[:],
                                              Act.Identity, bias=bcols["k"][1][m][:], scale=1.0)
                    proj_anchor.append(ka)

        # ============ phase C2: V projections + meanV (early, frees tpose sooner)
        with tc.tile_pool(name="ppsC2", bufs=2, space="PSUM") as pps, \
             tc.tile_pool(name="cw", bufs=1) as cw:
            bvb = cw.tile([128, FPC], f32)
            ps_b = pps.tile([128, 512], f32, tag="pp")
            nc.tensor.matmul(ps_b[:, 0:FPC], ones_row[:], bcols["v"][0][0:1, :],
                             start=True, stop=True)
            nc.scalar.copy(bvb[:], ps_b[:, 0:FPC])
            for j in range(8):
                psv = pps.tile([128, 512], f32, tag="pp")
                for i in range(6):
                    nc.tensor.matmul(psv[:, 0:FPC], ctxT[i][:, 128 * j:128 * j + 128],
                                     wts["v"][i][:], start=(i == 0), stop=(i == 5))
                nc.vector.tensor_tensor(V[j][:], psv[:, 0:FPC], bvb[:], Alu.add)

            # meanV [128, FPC] broadcast (for fully-masked rows)
            from concourse import bass_isa
            vsum = cw.tile([128, FPC], f32)
            nc.vector.tensor_tensor(vsum[:], V[0][:], V[1][:], Alu.add)
            for j in range(2, 8):
                nc.vector.tensor_tensor(vsum[:], vsum[:], V[j][:], Alu.add)
            meanv_b = cpool.tile([128, FPC], f32)
            nc.gpsimd.partition_all_reduce(meanv_b[:], vsum[:], channels=128,
                                           reduce_op=bass_isa.ReduceOp.add)
            nc.vector.tensor_scalar(meanv_b[:], meanv_b[:], 1.0 / S, None, op0=Alu.mult)

        # ---------------- main pools (open before C3 so heads 0-1 overlap it) ----
        import contextlib as _ctl
        mstack = _ctl.ExitStack()
        qkps = mstack.enter_context(tc.tile_pool(name="qkps", bufs=2, space="PSUM"))
        ptps = mstack.enter_context(tc.tile_pool(name="ptps", bufs=2, space="PSUM"))
        pvps = mstack.enter_context(tc.tile_pool(name="pvps", bufs=2, space="PSUM"))
        mp = mstack.enter_context(tc.tile_pool(name="mp", bufs=3))
        bigp = mstack.enter_context(tc.tile_pool(name="bigp", bufs=2))
        prp = mstack.enter_context(tc.tile_pool(name="prp", bufs=2))
        ptp = mstack.enter_context(tc.tile_pool(name="ptp", bufs=2))
        orp = mstack.enter_context(tc.tile_pool(name="orp", bufs=1))
        stp = mstack.enter_context(tc.tile_pool(name="stp", bufs=3))

        def neg_dma(rows, c0, c1, anchor):
            d = sc_d[:, rows[0]:rows[1], c0:c1]      # [6, 128, w]
            dd = _reorder(d, [1, 0, 2])              # [128(q), 6(h), w]
            s = neg[:, 0:c1 - c0]
            ss = bass.AP(s.tensor, s.offset, [s.ap[0], [0, 6], s.ap[1]])
            di = nc.gpsimd.dma_start(dd, ss)
            if anchor is not None:
                tile.add_dep_helper(di.ins, anchor.ins, sync=True, reason="defer NEG")

        def neg_blk_dma(h, anchor):
            d = sc_d[h, 512:1024, 512:1024]
            dd = bass.AP(d.tensor, d.offset,
                         [[1024, 128], [128 * 1024, 4], [1, 512]])
            s = neg[:, 0:512]
            ss = bass.AP(s.tensor, s.offset, [s.ap[0], [0, 4], s.ap[1]])
            di = nc.gpsimd.dma_start(dd, ss)
            if anchor is not None:
                tile.add_dep_helper(di.ins, anchor.ins, sync=True, reason="defer NEG")

        NEG_BAND = {1: ((0, 128), 128, 1024), 2: ((128, 256), 256, 1024),
                    3: ((256, 384), 0, 128), 4: ((256, 384), 384, 1024)}

        outall = orp.tile([128, 8 * FPC], f32, name="outall")

        def emit_head(h):
            m, po = h // 2, 64 * (h % 2)
            ssum = stp.tile([128, 8], f32, tag="ssum", name=f"ss{h}")
            pv_h = pvps.tile([128, 512], f32, tag="pv", name=f"pv{h}")
            first_exp = None

            bigsc = bigp.tile([128, 2048], f32, tag="bigsc", name=f"bsc{h}")
            for qc in range(8):
                if qc < 4:
                    ks, w = QC_WIN[qc]
                else:
                    ks, w = 0, 512
                lhs_q = QT[m][po:po + 64, 128 * qc:128 * qc + 128]

                if qc < 4:
                    sc = mp.tile([128, w], f32, tag=("sc3" if qc == 3 else f"sc{min(qc, 1)}"),
                                 bufs=(2 if qc == 3 else 3), name=f"sc{qc}_{h}")
                    scap = sc[:]
                    gt = gates[qc][:]
                else:
                    scap = bigsc[:, 512 * (qc - 4):512 * (qc - 4) + 512]
                    gt = gate_blk[:, 512 * (qc - 4):512 * (qc - 4) + 512]
                for n0 in range(0, w, 512):
                    nn = min(512, w - n0)
                    ps_qk = qkps.tile([128, nn], f32, tag="qk", name=f"qk{qc}_{h}_{n0}")
                    nc.tensor.matmul(ps_qk[:], lhs_q,
                                     KT[m][po:po + 64, ks + n0:ks + n0 + nn],
                                     start=True, stop=True)
                    if qc < 4:
                        nc.vector.tensor_tensor(sc[:, n0:n0 + nn], ps_qk[:],
                                                gates[qc][:, n0:n0 + nn], Alu.mult)
                    else:
                        nc.vector.tensor_tensor(scap, ps_qk[:], gt, Alu.mult)
                if use_amask:
                    nc.vector.tensor_tensor(scap, scap, am_b[:, ks:ks + w], Alu.add)
                if qc <= 2:
                    nc.gpsimd.affine_select(out=scap, in_=scap, compare_op=Alu.is_ge,
                                            fill=negreg, base=128 * qc - ks,
                                            channel_multiplier=1, pattern=[[-1, w]])
                    nc.gpsimd.affine_select(out=scap, in_=scap, compare_op=Alu.is_ge,
                                            fill=negreg, base=ks - 128 * qc + 1,
                                            channel_multiplier=-1, pattern=[[1, w]])
                elif qc == 3:
                    sc127 = sc[0:127, :]
                    nc.gpsimd.affine_select(out=sc127, in_=sc127, compare_op=Alu.is_ge,
                                            fill=negreg, base=384,
                                            channel_multiplier=1, pattern=[[-1, w]])
                    nc.gpsimd.affine_select(out=sc127, in_=sc127, compare_op=Alu.is_ge,
                                            fill=negreg, base=-383,
                                            channel_multiplier=-1, pattern=[[1, w]])
                elif qc in (5, 7):
                    half = bigsc[:, 1024 * ((qc - 4) // 2):1024 * ((qc - 4) // 2) + 1024]
                    nc.vector.copy_predicated(half, fpred2[:], neg[:])

                if qc < 4:
                    nc.sync.dma_start(sc_d[h, 128 * qc:128 * qc + 128, ks:ks + w], scap)

                def softmax_pv(qc2, scap2, ks2, w2):
                    nonlocal first_exp
                    pr = prp.tile([128, w2], f32r, tag="pr", name=f"pr{qc2}_{h}")
                    e = nc.scalar.activation(pr[:], scap2, Act.Exp,
                                             accum_out=ssum[:, qc2:qc2 + 1])
                    if first_exp is None:
                        first_exp = e
                    ps_pv = pv_h[:, 64 * qc2:64 * qc2 + 64]
                    nblk = w2 // 128
                    for g0 in range(0, nblk, 4):
                        gn = min(4, nblk - g0)
                        ps_t = ptps.tile([128, 512], f32r, tag="pt", name=f"pt{qc2}_{h}_{g0}")
                        for u in range(g0, g0 + gn):
                            nc.tensor.transpose(ps_t[:, 128 * (u - g0):128 * (u - g0) + 128],
                                                pr[:, 128 * u:128 * u + 128], ident_r[:])
                        prT = ptp.tile([128, 512], f32, tag="prT", name=f"prT{qc2}_{h}_{g0}")
                        if (qc2 + g0 // 4) % 2 == 0:
                            nc.scalar.copy(prT[:, 0:128 * gn], ps_t[:, 0:128 * gn])
                        else:
                            nc.vector.tensor_copy(prT[:, 0:128 * gn], ps_t[:, 0:128 * gn])
                        for u in range(g0, g0 + gn):
                            kc = ks2 // 128 + u
                            nc.tensor.matmul(ps_pv, prT[:, 128 * (u - g0):128 * (u - g0) + 128],
                                             V[kc][:, 64 * h:64 * h + 64],
                                             start=(u == 0), stop=(u == nblk - 1))

                if qc < 4:
                    softmax_pv(qc, scap, ks, w)
                elif qc in (5, 7):
                    for qc2 in (qc - 1, qc):
                        softmax_pv(qc2, bigsc[:, 512 * (qc2 - 4):512 * (qc2 - 4) + 512],
                                   0, 512)

            # deferred NEG-region DMAs, anchored on projection progress
            neg_blk_dma(h, proj_anchor[2 * h] if 2 * h < len(proj_anchor) else first_exp)
            if h in NEG_BAND:
                neg_dma(*NEG_BAND[h],
                        proj_anchor[2 * h + 1] if 2 * h + 1 < len(proj_anchor) else first_exp)

            # merged scores DMA for qc4-7
            d = sc_d[h, 512:1024, 0:512]
            dd = bass.AP(d.tensor, d.offset, [[1024, 128], [128 * 1024, 4], [1, 512]])
            s = bigsc[:, :]
            ss = bass.AP(s.tensor, s.offset, [s.ap[0], [512, 4], [1, 512]])
            nc.sync.dma_start(dd, ss)

            # batched normalization for all 8 qc
            w0 = stp.tile([128, 8], f32, tag="w0", name=f"w0{h}")
            nc.vector.tensor_scalar(w0[:], ssum[:], 0.0, None, op0=Alu.is_equal)
            ssafe = stp.tile([128, 8], f32, tag="ssafe", name=f"ssf{h}")
            nc.vector.tensor_tensor(ssafe[:], ssum[:], w0[:], Alu.add)
            rec = stp.tile([128, 8], f32, tag="rec", name=f"rc{h}")
            nc.vector.reciprocal(rec[:], ssafe[:])
            for qc in range(8):
                osl = outall[:, 384 * qc + 64 * h:384 * qc + 64 * h + 64]
                if h % 2 == 1:
                    nc.scalar.activation(osl, pv_h[:, 64 * qc:64 * qc + 64],
                                         Act.Identity, bias=0.0, scale=rec[:, qc:qc + 1])
                else:
                    nc.vector.tensor_scalar(osl, pv_h[:, 64 * qc:64 * qc + 64],
                                            rec[:, qc:qc + 1], None, op0=Alu.mult)
                if qc >= 4:
                    nc.vector.scalar_tensor_tensor(osl, in0=meanv_b[:, 64 * h:64 * h + 64],
                                                   scalar=w0[:, qc:qc + 1], in1=osl,
                                                   op0=Alu.mult, op1=Alu.add)

        # heads 0-1 need only QT[0]/KT[0] + V: emit before the m=1,2 projections
        emit_head(0)
        emit_head(1)

        # ============ phase C3: QT/KT projections for m=1,2
        with tc.tile_pool(name="ppsC3", bufs=2, space="PSUM") as pps:
            for m in [1, 2]:
                for n in range(2):
                    psq = pps.tile([128, 512], f32, tag="pp")
                    for i in range(6):
                        nc.tensor.matmul(psq[:], wts["q"][i][:, 128 * m:128 * m + 128],
                                         hsT[i][:, 512 * n:512 * n + 512],
                                         start=(i == 0), stop=(i == 5))
                    qa = nc.scalar.activation(QT[m][:, 512 * n:512 * n + 512], psq[:],
                                              Act.Identity, bias=bcols["q"][1][m][:], scale=0.125)
                    proj_anchor.append(qa)
                    psk = pps.tile([128, 512], f32, tag="pp")
                    for i in range(6):
                        nc.tensor.matmul(psk[:], wts["k"][i][:, 128 * m:128 * m + 128],
                                         ctxT[i][:, 512 * n:512 * n + 512],
                                         start=(i == 0), stop=(i == 5))
                    ka = nc.scalar.activation(KT[m][:, 512 * n:512 * n + 512], psk[:],
                                              Act.Identity, bias=bcols["k"][1][m][:], scale=1.0)
                    proj_anchor.append(ka)

        emit_head(2)

        # flush heads 0-2 output columns early
        d = out_d[:, 0:192]
        dd = bass.AP(d.tensor, d.offset, [[384, 128], [128 * 384, 8], [1, 192]])
        s = outall[:, 0:192]
        ss = bass.AP(s.tensor, s.offset, [s.ap[0], [384, 8], [1, 192]])
        nc.sync.dma_start(dd, ss)

        for h in range(3, HPC):
            emit_head(h)

        d = out_d[:, 192:384]
        dd = bass.AP(d.tensor, d.offset, [[384, 128], [128 * 384, 8], [1, 192]])
        s = outall[:, 192:384]
        ss = bass.AP(s.tensor, s.offset, [s.ap[0], [384, 8], [1, 192]])
        nc.sync.dma_start(dd, ss)

        mstack.close()
        prep2.close()
        prep1.close()

    nc.compile()
    return nc


class _Runner:
    """Compile once; run many times via PJRT shard_map across 8 cores."""

    def __init__(self, nc, n_cores=NCORES):
        import jax
        from concourse import bass2jax

        bass2jax.install_neuronx_cc_hook()
        self.nc = nc
        self.n_cores = n_cores
        partition_name = (nc.partition_id_tensor.name
                          if nc.partition_id_tensor is not None else None)
        in_names, out_names, out_avals, zero_outs = [], [], [], []
        for alloc in nc.m.functions[0].allocations:
            if not isinstance(alloc, mybir.MemoryLocationSet):
                continue
            name = alloc.memorylocations[0].name
            if alloc.kind == "ExternalInput":
                if name != partition_name:
                    in_names.append(name)
            elif alloc.kind == "ExternalOutput":
                out_names.append(name)
                shape = tuple(alloc.tensor_shape)
                dtype = mybir.dt.np(alloc.dtype)
                out_avals.append(jax.core.ShapedArray(shape, dtype))
                zero_outs.append(np.zeros(shape, dtype))
        self.dbg_name = nc.dbg_addr.name if nc.dbg_addr is not None else None
        if nc.dbg_addr is not None and nc.dbg_callbacks:
            raise RuntimeError("dbg callbacks not supported here")
        self.in_names = in_names          # includes dbg input if present
        self.out_names = out_names
        self.zero_outs = zero_outs
        n_params, n_outs = len(in_names), len(out_names)
        all_in_names = list(in_names) + list(out_names)
        if partition_name is not None:
            all_in_names.append(partition_name)
        donate = tuple(range(n_params, n_params + n_outs))

        devices = jax.devices()[:n_cores]
        assert len(devices) == n_cores
        self.mesh = bass2jax.Mesh(np.asarray(devices), ("core",))
        self.sharding = jax.sharding.NamedSharding(
            self.mesh, bass2jax.PartitionSpec("core"))
        in_specs = (bass2jax.PartitionSpec("core"),) * (n_params + n_outs)
        out_specs = (bass2jax.PartitionSpec("core"),) * n_outs

        def _body(*args):
            operands = list(args)
            if partition_name is not None:
                operands.append(bass2jax.partition_id_tensor())
            outs = bass2jax._bass_exec_p.bind(
                *operands,
                out_avals=tuple(out_avals),
                in_names=tuple(all_in_names),
                out_names=tuple(out_names),
                lowering_input_output_aliases=(),
                sim_require_finite=True,
                sim_require_nnan=True,
                nc=nc,
            )
            return tuple(outs)

        self.fn = jax.jit(
            bass2jax.shard_map(_body, mesh=self.mesh, in_specs=in_specs,
                               out_specs=out_specs, check_rep=False),
            donate_argnums=donate, keep_unused=True)
        self._jax = jax

    def stage_inputs(self, in_maps):
        jax = self._jax
        if self.dbg_name is not None:
            in_maps = [{**m, self.dbg_name: np.zeros((1, 2), np.uint32)} for m in in_maps]
        staged = []
        for name in self.in_names:
            g = np.concatenate([np.asarray(m[name]) for m in in_maps], axis=0)
            staged.append(jax.device_put(g, self.sharding))
        return staged

    def stage_zeros(self):
        jax = self._jax
        return [jax.device_put(np.concatenate([z] * self.n_cores, axis=0), self.sharding)
                for z in self.zero_outs]

    def run_staged(self, staged, zeros):
        return self.fn(*staged, *zeros)

    def run(self, in_maps):
        staged = self.stage_inputs(in_maps)
        zeros = self.stage_zeros()
        outs = self.run_staged(staged, zeros)
        results = []
        for c in range(self.n_cores):
            d = {}
            for i, name in enumerate(self.out_names):
                arr = np.asarray(outs[i])
                per = arr.shape[0] // self.n_cores
                d[name] = arr[c * per:(c + 1) * per]
            results.append(d)
        return results


_RUNNERS = {}


def _get_runner(use_amask: bool):
    if use_amask not in _RUNNERS:
        nc = _build(use_amask)
        _RUNNERS[use_amask] = _Runner(nc)
    return _RUNNERS[use_amask]


def _make_in_maps(hidden_states, context, attention_mask, attention_gate,
                  vis_position, Wq, bq, Wk, bk, Wv, bv, feat_len):
    f = np.float32
    in_maps = []
    feat_len = np.asarray(feat_len).astype(np.int32)
    for c in range(NCORES):
        b, hg = c // 2, c % 2
        fs = slice(FPC * hg, FPC * hg + FPC)
        in_maps.append({
            "hs": np.ascontiguousarray(hidden_states[b], f),
            "ctx": np.ascontiguousarray(context[b], f),
            "vis": np.ascontiguousarray(vis_position[b], f),
            "gate": np.ascontiguousarray(attention_gate[b, 0], f),
            "amask": np.ascontiguousarray(attention_mask[b, 0, 0][None, :], f),
            "Wq": np.ascontiguousarray(Wq[fs], f),
            "Wk": np.ascontiguousarray(Wk[fs], f),
            "Wv": np.ascontiguousarray(Wv[fs], f),
            "bq": np.ascontiguousarray(bq[fs][None, :], f),
            "bk": np.ascontiguousarray(bk[fs][None, :], f),
            "bv": np.ascontiguousarray(bv[fs][None, :], f),
            "fl": feat_len[b].reshape(1, 1),
        })
    return in_maps


def kernel(hidden_states, context, attention_mask, attention_gate, vis_position,
           Wq, bq, Wk, bk, Wv, bv, feat_len, prox_pos):
    assert int(prox_pos) == PROX, f"kernel compiled for prox_pos={PROX}"
    hidden_states = np.asarray(hidden_states)
    context = np.asarray(context)
    attention_mask = np.asarray(attention_mask)
    attention_gate = np.asarray(attention_gate)
    vis_position = np.asarray(vis_position)
    Wq, bq = np.asarray(Wq), np.asarray(bq)
    Wk, bk = np.asarray(Wk), np.asarray(bk)
    Wv, bv = np.asarray(Wv), np.asarray(bv)

    use_amask = bool(np.any(attention_mask))
    runner = _get_runner(use_amask)
    in_maps = _make_in_maps(hidden_states, context, attention_mask,
                            attention_gate, vis_position, Wq, bq, Wk, bk, Wv, bv,
                            feat_len)
    results = runner.run(in_maps)

    out = np.empty((B, S, H), np.float32)
    scores = np.empty((B, NH, S, S), np.float32)
    for c in range(NCORES):
        b, hg = c // 2, c % 2
        scores[b, HPC * hg:HPC * hg + HPC] = results[c]["scores"]
        out[b, :, FPC * hg:FPC * hg + FPC] = results[c]["out"]
    return out, scores
